# revision 1
# baseline (speedup 1.0000x reference)
"""Int4 quantized embedding lookup on 8 Trainium2 NeuronCores.

Strategy: data-parallel over tokens (16384 tokens -> 2048/core). One merged
table [N, 1152] u8 = repacked nibbles (1024 B) + per-row [scale | -zp*scale]
f32 (128 B) is replicated to every core's HBM. Each core gathers its tokens'
rows with one indirect DMA per 128-token tile, unpacks nibbles on DVE (u32
shift/mask), dequantizes per group as q*s + b in one pass per group — split
between DVE (tensor_scalar mult+add, per-partition AP scalars) and ACT
(activation Identity with per-partition scale/bias) — and writes its
[2048, 2048] f32 output slab contiguously.

Host-side prep (one-time, off the hot path): nibble repack so the device
unpack lands in true d-order (new byte j = q[d=j]<<4 | q[d=1024+j]), and the
fused [scale | -zp*scale] tail so dequant needs no on-device zp math.
"""

import numpy as np

N_EMB = 128000
EMB_DIM = 2048
PACKED = EMB_DIM // 2  # 1024 packed bytes per row
NGROUP = 16
GROUP = 128
ROW = PACKED + 2 * NGROUP * 4  # 1152 bytes per merged-table row
B, S = 4, 4096
NCORES = 8
TOK = B * S
TPC = TOK // NCORES  # tokens per core = 2048
P = 128
NTILES = TPC // P  # 16
DVE_GROUPS = 10  # groups [0, DVE_GROUPS) on DVE, rest on ACT
STORE_MERGE = 2  # tiles per output store DMA
MAX_WAITS = 1

_cached = {}


def _split_sync_waits(nc, mybir, max_waits=MAX_WAITS):
    """neuronxcc walrus allows very few sync-wait slots per instruction; Tile
    emits up to one wait per logical proc. Split overflow waits onto preceding
    same-engine NOPs (sequential waits == AND)."""
    ctr = 0
    for f in nc.m.functions:
        for bb in f.blocks:
            il = bb.instructions
            out = []
            changed = False
            for inst in il:
                si = inst.sync_info
                waits = list(si.on_wait) if si is not None else []
                if len(waits) > max_waits:
                    over = waits[max_waits:]
                    for j in range(0, len(over), max_waits):
                        ctr += 1
                        nop = mybir.InstNoOp(
                            name=f"wsplit-{ctr}",
                            sync_info=mybir.SyncInfo(
                                on_wait=over[j : j + max_waits], on_update=[]
                            ),
                            bass_nofuse=True,
                            engine=inst.engine,
                        )
                        nc.register_instruction(nop)
                        out.append(nop)
                    inst.sync_info = mybir.SyncInfo(
                        on_wait=waits[:max_waits], on_update=list(si.on_update)
                    )
                    changed = True
                out.append(inst)
            if changed:
                il[:] = out
    return ctr


def _build_bass(repeat=1, bufs=3, dve_groups=DVE_GROUPS):
    """repeat>1 re-executes the whole workload K times writing the same
    outputs (idempotent) — used by test.py to slope-time the NEFF."""
    import concourse.bass as bass
    import concourse.mybir as mybir
    import concourse.tile as tile

    nc = bass.Bass("TRN2")
    ids = nc.dram_tensor("ids", [TPC], mybir.dt.int32, kind="ExternalInput")
    table = nc.dram_tensor("table", [N_EMB, ROW], mybir.dt.uint8, kind="ExternalInput")
    out = nc.dram_tensor("out", [TPC, EMB_DIM], mybir.dt.float32, kind="ExternalOutput")

    u32 = mybir.dt.uint32
    f32 = mybir.dt.float32
    u8 = mybir.dt.uint8

    with tile.TileContext(nc) as tc:
        with (
            tc.tile_pool(name="idsp", bufs=1) as idsp,
            tc.tile_pool(name="gath", bufs=NTILES) as gp,
            tc.tile_pool(name="work", bufs=bufs) as wp,
        ):
            # tile-0 ids load is tiny and on Pool (earliest-booting engine)
            # so gather 0 can issue ASAP; the rest load via idle HWDGE
            ids0 = idsp.tile([P, 1], mybir.dt.int32, tag="ids0")
            nc.gpsimd.dma_start(
                ids0[:], ids.ap()[0:P].rearrange("(p o) -> p o", o=1)
            )
            ids_sb = idsp.tile([P, NTILES - 1], mybir.dt.int32, tag="idsr")
            nc.sync.dma_start(
                ids_sb[:], ids.ap()[P:TPC].rearrange("(t p) -> p t", p=P)
            )

            for rep in range(repeat):
                # front-load all gathers: Q7 descriptor generation is ~1us
                # per indirect DMA and would otherwise pace the pipeline
                pks = []
                for t in range(NTILES):
                    idx = ids0[:, 0:1] if t == 0 else ids_sb[:, t - 1 : t]
                    pk = gp.tile([P, ROW], u8, tag="pk")
                    nc.gpsimd.indirect_dma_start(
                        out=pk[:],
                        out_offset=None,
                        in_=table.ap(),
                        in_offset=bass.IndirectOffsetOnAxis(ap=idx, axis=0),
                    )
                    pks.append(pk)

                for pair in range(NTILES // STORE_MERGE):
                    # STORE_MERGE tiles' dequant lands in one wide tile so the
                    # store ships STORE_MERGE MB in one DMA (amortizes per-DMA
                    # overheads)
                    w2 = wp.tile([P, STORE_MERGE * EMB_DIM], f32, tag="w")
                    for sub in range(STORE_MERGE):
                        t = STORE_MERGE * pair + sub
                        pk = pks[t]
                        wofs = sub * EMB_DIM
                        sz = pk[:, PACKED:ROW].bitcast(f32)  # [P,32]=[s16|b16]

                        # unpack nibbles with u32 shift/mask (4 B per lane-op)
                        q = wp.tile([P, EMB_DIM], u8, tag="q")
                        pk32 = pk[:, 0:PACKED].bitcast(u32)
                        q32 = q[:].bitcast(u32)
                        half = PACKED // 4  # 256 u32 words
                        nc.vector.tensor_scalar(
                            q32[:, 0:half],
                            pk32,
                            4,
                            0x0F0F0F0F,
                            mybir.AluOpType.logical_shift_right,
                            mybir.AluOpType.bitwise_and,
                        )
                        nc.vector.tensor_scalar(
                            q32[:, half : 2 * half],
                            pk32,
                            0x0F0F0F0F,
                            None,
                            mybir.AluOpType.bitwise_and,
                        )

                        # one fused q*s + b per group; DVE and ACT split groups
                        for g in range(NGROUP):
                            c0 = g * GROUP
                            if g < dve_groups:
                                nc.vector.tensor_scalar(
                                    w2[:, wofs + c0 : wofs + c0 + GROUP],
                                    q[:, c0 : c0 + GROUP],
                                    sz[:, g : g + 1],
                                    sz[:, NGROUP + g : NGROUP + g + 1],
                                    mybir.AluOpType.mult,
                                    mybir.AluOpType.add,
                                )
                            else:
                                nc.scalar.activation(
                                    out=w2[:, wofs + c0 : wofs + c0 + GROUP],
                                    in_=q[:, c0 : c0 + GROUP],
                                    func=mybir.ActivationFunctionType.Identity,
                                    bias=sz[:, NGROUP + g : NGROUP + g + 1],
                                    scale=sz[:, g : g + 1],
                                )

                    # one merged store: DRAM rows [SM*128*pair, +SM*128)
                    out3 = bass.AP(
                        out.ap().tensor,
                        STORE_MERGE * pair * P * EMB_DIM,
                        [[EMB_DIM, P], [P * EMB_DIM, STORE_MERGE], [1, EMB_DIM]],
                    )
                    w3 = w2[:].rearrange("p (h d) -> p h d", d=EMB_DIM)
                    nc.sync.dma_start(out3, w3)

    _split_sync_waits(nc, mybir)
    return nc


def _prep_tables(weight_packed, scale, zero_point):
    wp = np.asarray(weight_packed, dtype=np.uint8)
    hi = (wp >> 4).astype(np.uint8)
    lo = (wp & 15).astype(np.uint8)
    q = np.empty((N_EMB, EMB_DIM), np.uint8)
    q[:, 0::2] = hi
    q[:, 1::2] = lo
    repacked = (q[:, :PACKED] << 4) | q[:, PACKED:]
    sc = np.asarray(scale, dtype=np.float32)
    zp = np.asarray(zero_point, dtype=np.float32)
    fused = np.concatenate([sc, -(zp * sc)], axis=1).astype(np.float32)
    table = np.concatenate([repacked, fused.view(np.uint8)], axis=1)
    return np.ascontiguousarray(table)


def kernel(input_ids, weight_packed, scale, zero_point):
    from concourse import bass_utils

    if "nc" not in _cached:
        _cached["nc"] = _build_bass()
    nc = _cached["nc"]

    table = _prep_tables(weight_packed, scale, zero_point)
    ids = np.ascontiguousarray(np.asarray(input_ids).reshape(-1).astype(np.int32))

    in_maps = [
        {"ids": ids[c * TPC : (c + 1) * TPC], "table": table} for c in range(NCORES)
    ]
    res = bass_utils.run_bass_kernel_spmd(nc, in_maps, core_ids=list(range(NCORES)))
    out = np.concatenate([res.results[c]["out"] for c in range(NCORES)], axis=0)
    return out.reshape(B, S, EMB_DIM)

